# revision 1
# baseline (speedup 1.0000x reference)
"""4D circular cross-correlation (qcd_ml C_Convolution, k=3, nd=4) on 8 TRN2 cores.

Math: out[o, x,y,z,t, s,c] = b[o] + sum_{i, ax,ay,az,at} W[i,o,ax,ay,az,at]
                                   * U[i, x+ax-1, y+ay-1, z+az-1, t+at-1, s,c]
(all site indices circular). U complex64 (4,16,16,16,32,4,3), W complex64
(4,4,3,3,3,3), b complex64 (4,).

Device mapping (per core, T sharded 8-way with +-1 halos prepared on host):
  - contraction (matmul partition) dim = (reim_in 2, C_in 4, X 16) = 128
  - output (PSUM partition) dim       = (reim_out 2, C_out 4, X0 16) = 128
  - X offsets (ax) live inside the stationary 128x128 matrices, circularly
    banded in (x, x0); complex arithmetic is the 2x2 [[Wr, Wi], [-Wi, Wr]]
    block over the reim axes.
  - The T offsets (at) are removed by a host-side Winograd F(4,3) transform
    along t: the 4 local t outputs form ONE tile whose 6-point input window
    is exactly the t-halo slab; U becomes 6 phases (B^T d), weights become
    G W (6 phases); the device accumulates 9 (ay,az) offsets per phase into
    PSUM and combines the 6 phase results with A^T on the vector engine.
  - moving free dim = (y-pair 2, z 16, spin*color 12) = 384 <= 512 (one
    PSUM bank per phase).
  - y,z circular handled by host padding to 18; t halo from neighbor T-slab.

Performance shape (measured, 8x TRN2, ~92-93us):
  - All matmul data is fp16 (rel_err 9.9e-4 vs 2e-2 gate): full PE rate
    (1 col/cycle @2.4GHz), LoadStationary ~97ns hides under the 163ns
    matmul, input DMA halves vs f32. fp32r LS (~195ns) was the original
    pipeline bottleneck.
  - 432 matmuls x 163ns = 70.4us solid PE streak at the instruction floor.
  - Fixed overheads: ~8.7us runtime/DGE entry before the first DMA byte,
    ~3.3us exit barrier. Warm-up matmuls bridge the PE through the input
    DMA wait so the p-state is at full clock when real work starts.
  - PSUM->SBUF drains ride the Act engine; the A^T combine (11 DVE ops per
    y-pair) hides under the next pair's matmuls; phases run in combine
    order (1,2,0,3,4,5); t-major output tiles let each pair's store split
    into two halves that overlap the combine tail.
  - Single SP DMA queue, explicitly consumption-ordered (multi-queue
    splits share the same ~420GB/s and starve the small wstat stream).
"""

import os
import sys
import itertools
import numpy as np

for _p in ("/opt/trn_rl_repo",):
    if _p not in sys.path and os.path.isdir(_p):
        sys.path.insert(0, _p)

C_IN, C_OUT = 4, 4
X = Y = Z = 16
T = 32
SC = 12  # spin*color
NCORES = 8
TLOC = T // NCORES          # 4 = one F(4,3) output tile
NPH = 6                     # Winograd F(4,3) phases
YPAD, ZPAD = Y + 2, Z + 2   # 18
UH_ROWS = 10                # y_pad rows per half tile (0..9 / 8..17)
OFF9 = list(itertools.product(range(3), repeat=2))  # (ay, az)
FREE = 2 * Z * SC           # 384, one chunk = (y-pair, z, sc) per phase

# 16-bit data path: halves input DMA and, critically, halves LoadStationary
# time (fp32r LS ~195ns > 384-col matmul ~160ns made the PE pipeline
# LS-bound; 16-bit LS ~97ns makes it matmul-bound). fp16 over bf16: same PE
# rate (1 col/cycle), 4x finer mantissa. Values are small (|U~|<=34,
# |wstat|<=4, products accumulate in f32 PSUM) so fp16 range is safe.
# Measured rel_err: bf16 7.9e-3, fp32r 4.7e-4 (gate 2e-2).
CONV_DT = os.environ.get("CONV_DT", "fp16")
USE_16BIT = CONV_DT in ("fp16", "bf16")

# Winograd F(4,3), points [0,1,-1,2,-2,inf] (correlation form:
# out[r] = sum_k g[k] d[r+k], r=0..3, d = U[t0-1 .. t0+4]).
BT = np.array([
    [4, 0, -5, 0, 1, 0],
    [0, -4, -4, 1, 1, 0],
    [0, 4, -4, -1, 1, 0],
    [0, -2, -1, 2, 1, 0],
    [0, 2, -1, -2, 1, 0],
    [0, 4, 0, -5, 0, 1]], np.float64)
G = np.array([
    [1 / 4, 0, 0],
    [-1 / 6, -1 / 6, -1 / 6],
    [-1 / 6, 1 / 6, -1 / 6],
    [1 / 24, 1 / 12, 1 / 6],
    [1 / 24, -1 / 12, 1 / 6],
    [0, 0, 1]], np.float64)
# A^T = [[1,1,1,1,1,0],
#        [0,1,-1,2,-2,0],
#        [0,1,1,4,4,0],
#        [0,1,-1,8,-8,1]]  -- applied on the device (DVE).


def _np_dt():
    if CONV_DT == "fp16":
        return np.dtype(np.float16)
    if CONV_DT == "bf16":
        import ml_dtypes
        return np.dtype(ml_dtypes.bfloat16)
    return np.dtype(np.float32)


def _prep_u_shards(U):
    """U complex (4,16,16,16,32,4,3) -> per-core arrays (fp16/bf16/f32)
    [128, NPH, YPAD, ZPAD, SC] of the t-Winograd-transformed field.

    Phase-major layout: a (phase, row-chunk) DMA slice is contiguous per
    partition, so the device stream can start on phase 1's first rows
    (~220KB) instead of waiting for all phases of the first rows (1.3MB)."""
    dt = _np_dt()
    Ur = np.stack([U.real, U.imag], axis=0).astype(np.float32)  # (2,4,X,Y,Z,T,4,3)
    Ur = Ur.reshape(2, C_IN, X, Y, Z, T, SC)
    Up = np.pad(Ur, ((0, 0), (0, 0), (0, 0), (1, 1), (1, 1), (0, 0), (0, 0)),
                mode="wrap")  # (2,4,16,18,18,32,12)
    shards = []
    for k in range(NCORES):
        t0 = k * TLOC
        tidx = np.arange(t0 - 1, t0 + 5) % T        # 6-point window
        d = np.take(Up, tidx, axis=5)               # (2,4,16,18,18,6,12)
        m = np.einsum("pk,rixyzks->rixpyzs", BT,
                      d.astype(np.float64)).astype(dt)
        m = m.reshape(128, NPH, YPAD, ZPAD, SC)
        shards.append(np.ascontiguousarray(m))
    return shards


def _prep_wstat(W):
    """W complex (4,4,3,3,3,3) -> [128, NPH*9, 128] float32 stationary stack.

    For phase p and (ay,az): Wg[p][i,o,ax,ay,az] = sum_at G[p,at] W[..,at];
    band in (x,x0): ax = (x - x0 + 1) mod 16 in {0,1,2};
    ri block M = [[Wr, Wi], [-Wi, Wr]] (columns riO: out_r, out_i).
    """
    Wc = np.ascontiguousarray(W).astype(np.complex128)
    Wg = np.einsum("pk,ioxyzk->pioxyz", G.astype(np.complex128), Wc)
    Wg = Wg.astype(np.complex64)                    # (6,4,4,3,3,3)
    stat = np.zeros((2, C_IN, X, NPH * 9, 2, C_OUT, X), _np_dt())
    for ph in range(NPH):
        for aidx, (ay, az) in enumerate(OFF9):
            widx = ph * 9 + aidx
            for ax in range(3):
                wr = Wg[ph, :, :, ax, ay, az].real
                wi = Wg[ph, :, :, ax, ay, az].imag
                for x0 in range(X):
                    x = (x0 + ax - 1) % X
                    stat[0, :, x, widx, 0, :, x0] = wr
                    stat[1, :, x, widx, 0, :, x0] = -wi
                    stat[0, :, x, widx, 1, :, x0] = wi
                    stat[1, :, x, widx, 1, :, x0] = wr
    return np.ascontiguousarray(stat.reshape(128, NPH * 9, 128))


def _assemble(results, b):
    """results[k]["out"]: [128, Y//2, TLOC, 2, Z, SC] -> complex (4,16,16,16,32,4,3)."""
    out = np.empty((C_OUT, X, Y, Z, T, SC), np.complex64)
    for k in range(NCORES):
        r = np.asarray(results[k]["out"], np.float32).reshape(
            2, C_OUT, X, Y // 2, TLOC, 2, Z, SC)
        # (ri, o, x, g, t, y2, z, s) -> (ri, o, x, g, y2, z, t, s)
        r = r.transpose(0, 1, 2, 3, 5, 6, 4, 7).reshape(
            2, C_OUT, X, Y, Z, TLOC, SC)
        out[:, :, :, :, k * TLOC:(k + 1) * TLOC, :] = r[0] + 1j * r[1]
    out += np.asarray(b, np.complex64).reshape(C_OUT, 1, 1, 1, 1, 1)
    return np.ascontiguousarray(out.reshape(C_OUT, X, Y, Z, T, 4, 3))


def _build_nc():
    import concourse.mybir as mybir
    from concourse import bacc, tile
    from contextlib import ExitStack

    f32 = mybir.dt.float32
    _dt16 = {"fp16": mybir.dt.float16, "bf16": mybir.dt.bfloat16}
    mm_dt = _dt16.get(CONV_DT, mybir.dt.float32r)
    out_dt = _dt16.get(CONV_DT, f32)
    tmp_dt = f32  # DVE temps stay f32: 16-bit gave no DVE speedup, cost accuracy
    AluOp = mybir.AluOpType

    WCOLS = NPH * 9 * 128              # 6912
    UCOLS = UH_ROWS * ZPAD * NPH * SC  # 12960

    nc = bacc.Bacc()
    # Fine-grained consumption-ordered input streaming: one full U~ tile
    # filled by disjoint row-slice DMAs (no y duplication), wstat split per
    # phase. Pair 0's phase-0 data (ws[0] + rows 0..5 of phase 0) lands after
    # ~1.2 MB, so the PE starts within a few us; dependencies are tracked at
    # address level, so each matmul only waits for the slices it reads.
    w_dram = nc.declare_dram_parameter("wstat", [128, NPH, 9, 128], mm_dt, isOutput=False)
    u_dram = nc.declare_dram_parameter("u", [128, NPH, YPAD, ZPAD, SC], mm_dt, isOutput=False)
    # t-major output layout: each A^T result row ow(r) is a CONTIGUOUS
    # [2y,Z,SC] block, and the per-pair store splits into two contiguous
    # halves so the first half overlaps the tail of the DVE combine.
    o_dram = nc.declare_dram_parameter("out", [128, Y // 2, TLOC, 2, Z, SC], out_dt, isOutput=True)

    with tile.TileContext(nc) as tc, ExitStack() as ctx:
        ipool = ctx.enter_context(tc.tile_pool(name="inp", bufs=1))
        opool = ctx.enter_context(tc.tile_pool(name="o", bufs=2))
        tpool = ctx.enter_context(tc.tile_pool(name="tmp", bufs=1))
        ppool = ctx.enter_context(tc.tile_pool(name="psum", bufs=7, space="PSUM"))
        wpool = ctx.enter_context(tc.tile_pool(name="warmp", bufs=1, space="PSUM"))

        wt = ipool.tile([128, NPH, 9, 128], mm_dt, tag="w")
        ufull = ipool.tile([128, NPH, YPAD, ZPAD, SC], mm_dt, tag="u")
        # All slices are CONTIGUOUS per partition. Single SP queue:
        # aggregate DMA bandwidth is shared across queues (measured
        # ~420GB/s) and multi-queue arbitration starves small streams;
        # explicit consumption order on one queue beats it. Phase-major
        # U layout: the first matmuls need only wt[1][0:3] + phase 1 rows
        # 0:4 (~320KB), so the stream starts ~4us earlier than with all
        # phases interleaved per row. Pair 0's phase chunks then arrive
        # ~0.7us apart vs 1.44us/phase compute; later pairs get their two
        # new rows per phase (6 x 110KB per pair) far ahead of use.
        nc.sync.dma_start(wt[:, 1, 0:3], w_dram[:, 1, 0:3])
        nc.sync.dma_start(ufull[:, 1, 0:4], u_dram[:, 1, 0:4])
        nc.sync.dma_start(wt[:, 1, 3:9], w_dram[:, 1, 3:9])
        nc.sync.dma_start(ufull[:, 2, 0:4], u_dram[:, 2, 0:4])
        nc.sync.dma_start(wt[:, 2:3], w_dram[:, 2:3])
        nc.sync.dma_start(ufull[:, 0, 0:4], u_dram[:, 0, 0:4])
        nc.sync.dma_start(wt[:, 0:1], w_dram[:, 0:1])
        for ph in (3, 4, 5):
            nc.sync.dma_start(ufull[:, ph, 0:4], u_dram[:, ph, 0:4])
            nc.sync.dma_start(wt[:, ph:ph + 1], w_dram[:, ph:ph + 1])
        for g in range(1, Y // 2):
            r0, r1 = 2 * g + 2, 2 * g + 4
            for ph in (1, 2, 0, 3, 4, 5):
                nc.sync.dma_start(ufull[:, ph, r0:r1], u_dram[:, ph, r0:r1])

        # PE warm-up: dummy matmuls on a zeroed scratch tile while the input
        # DMA streams (~5us of otherwise-idle PE time). The PE p-state ramps
        # to full clock only after ~3us of continuous busy; warming it here
        # means the first real matmuls run at full rate instead of ramping
        # mid-stream. Results go to a scratch PSUM tile and are discarded.
        # 8 x ~320ns cadence ends ~10.2us, at the typical arrival of the
        # first phase chunk (DMA start itself jitters 8.7-9.7us run to
        # run). A residual gap of up to ~2us does not reset the p-state
        # (measured: 1.4us gap kept full clock, 3.0-3.3us gaps did not).
        warm = ipool.tile([128, FREE], mm_dt, tag="warm")
        nc.gpsimd.memset(warm[:], 0.0)
        wps = wpool.tile([128, FREE], f32)
        for _ in range(15):
            nc.tensor.matmul(wps[:], warm[:, 0:128], warm[:],
                             start=True, stop=True)

        def stt(out_ap, sb_in, scalar, ps_or_sb):
            # out = (sb_in * scalar) +/- second operand, via scalar_tensor_tensor
            nc.vector.scalar_tensor_tensor(
                out_ap, in0=sb_in, scalar=scalar, in1=ps_or_sb,
                op0=AluOp.mult, op1=AluOp.add)

        YG = 2  # one out-DMA per y-pair: short tail, early PSUM drain
        for g in range(Y // YG):
            ot = opool.tile([128, TLOC, YG, Z, SC], out_dt)
            for pair in range(YG // 2):
                y = g * YG + pair * 2               # even; pair (y, y+1)
                # Phase order matches the combine's consumption order
                # (m1c needs ph1 first, bt_ ph2, t0a ph0, ...), so the DVE
                # chain starts ~1.4us earlier relative to this pair's last
                # matmul — less combine spillover past the final matmul.
                # The LAST pair ends on ph0 instead: then only t0a/ow(0)
                # are gated on the kernel's final matmul and the t=2,3
                # half-store completes before it.
                last = (g == Y // YG - 1)
                pts = [None] * NPH
                for ph in ((1, 2, 3, 4, 5, 0) if last else (1, 2, 0, 3, 4, 5)):
                    pt = ppool.tile([128, FREE], f32)
                    for aidx, (ay, az) in enumerate(OFF9):
                        rhs = ufull[:, ph, y + ay: y + ay + 2, az: az + Z, :]
                        nc.tensor.matmul(
                            pt[:],
                            wt[:, ph, aidx, :],
                            rhs,
                            start=(aidx == 0),
                            stop=(aidx == 8),
                        )
                    pts[ph] = pt
                # A^T combine; every DVE op reads at most one PSUM operand.
                # b=m1+m2, a=m1-m2, u=m3+m4, s=m3-m4
                # t0=m0+b+u; t1=a+2s; t2=b+4u; t3=a+8s+m5
                # Ordered so PSUM banks m1,m2,m0,m3,m4 free as early as
                # possible (the next pair's matmuls reuse them).
                # PSUM->SBUF copies on the Activation engine: they come off
                # the DVE critical path and overlap the DVE combines.
                m1c = tpool.tile([128, FREE], tmp_dt, tag="m1c")
                nc.scalar.copy(m1c[:], pts[1][:])
                bt_ = tpool.tile([128, FREE], tmp_dt, tag="bt")
                nc.vector.tensor_add(bt_[:], m1c[:], pts[2][:])
                t0a = tpool.tile([128, FREE], tmp_dt, tag="t0a")
                if not last:
                    nc.vector.tensor_add(t0a[:], bt_[:], pts[0][:])
                m3c = tpool.tile([128, FREE], tmp_dt, tag="m3c")
                nc.scalar.copy(m3c[:], pts[3][:])
                ut_ = tpool.tile([128, FREE], tmp_dt, tag="ut")
                nc.vector.tensor_add(ut_[:], m3c[:], pts[4][:])
                a_ = tpool.tile([128, FREE], tmp_dt, tag="at")
                nc.vector.scalar_tensor_tensor(
                    a_[:], in0=m1c[:], scalar=2.0, in1=bt_[:],
                    op0=AluOp.mult, op1=AluOp.subtract)
                s_ = tpool.tile([128, FREE], tmp_dt, tag="st")
                nc.vector.scalar_tensor_tensor(
                    s_[:], in0=m3c[:], scalar=2.0, in1=ut_[:],
                    op0=AluOp.mult, op1=AluOp.subtract)
                # writes into ot: contiguous (y2, z, sc) block at t=r
                def ow(r):
                    return ot[:, r]
                # t=0,1 first so their half-store can fire early; t3a before
                # ow(2) so ow(3) — the only op gated on phase 5's last
                # matmul — issues as soon as possible. For the last pair
                # (phase 0 computed last) every op except ow(0) is ordered
                # to clear the DVE queue before the kernel's final matmul
                # lands, so ow(0) = bu + ps0 starts the moment its
                # semaphore fires; the t=2,3 half-store is enqueued first.
                if not last:
                    nc.vector.tensor_add(ow(0), t0a[:], ut_[:])
                stt(ow(1), s_[:], 2.0, a_[:])
                t3a = tpool.tile([128, FREE], tmp_dt, tag="t3a")
                nc.vector.scalar_tensor_tensor(
                    t3a[:], in0=s_[:], scalar=8.0, in1=a_[:],
                    op0=AluOp.mult, op1=AluOp.add)
                if last:
                    nc.vector.tensor_add(ow(3), t3a[:], pts[5][:])
                    stt(ow(2), ut_[:], 4.0, bt_[:])
                    nc.vector.tensor_add(t0a[:], bt_[:], ut_[:])
                    nc.vector.tensor_add(ow(0), t0a[:], pts[0][:])
                else:
                    stt(ow(2), ut_[:], 4.0, bt_[:])
                    nc.vector.tensor_add(ow(3), t3a[:], pts[5][:])
            # Contiguous half-stores per pair: the earlier-finished half
            # fires first, overlapping the rest of the combine. The last
            # pair trails only the single t=0 row behind its final matmul.
            if last:
                nc.sync.dma_start(o_dram[:, g, 2:4], ot[:, 2:4])
                nc.sync.dma_start(o_dram[:, g, 1:2], ot[:, 1:2])
                nc.sync.dma_start(o_dram[:, g, 0:1], ot[:, 0:1])
            else:
                nc.sync.dma_start(o_dram[:, g, 0:2], ot[:, 0:2])
                nc.sync.dma_start(o_dram[:, g, 2:4], ot[:, 2:4])

    # Bacc defers register allocation and sync-wait splitting to finalize();
    # run_bass_via_pjrt serializes the module as-is, so finalize here.
    nc.finalize()
    return nc


_NC_CACHE = None
LAST_RUN = None  # BassKernelResults of the most recent device run (for test.py)


def kernel(U, W, b):
    global _NC_CACHE, LAST_RUN
    shards = _prep_u_shards(np.asarray(U))
    wstat = _prep_wstat(np.asarray(W))

    if os.environ.get("CONV_EMULATE", "0") == "1":
        results = _emulate(shards, wstat)
    else:
        from concourse.bass_utils import run_bass_kernel_spmd
        if _NC_CACHE is None:
            _NC_CACHE = _build_nc()
        wr = np.ascontiguousarray(wstat.reshape(128, NPH, 9, 128))
        in_maps = [{"wstat": wr, "u": u} for u in shards]
        trace = os.environ.get("CONV_TRACE", "0") == "1"
        LAST_RUN = run_bass_kernel_spmd(
            _NC_CACHE, in_maps, core_ids=list(range(NCORES)), trace=trace)
        results = LAST_RUN.results
    return _assemble(results, np.asarray(b))


def _emulate(shards, wstat):
    """Host-side emulation of the device program (float64 accumulate)."""
    AT = np.array([
        [1, 1, 1, 1, 1, 0],
        [0, 1, -1, 2, -2, 0],
        [0, 1, 1, 4, 4, 0],
        [0, 1, -1, 8, -8, 1]], np.float64)
    results = []
    for u in shards:
        out = np.zeros((128, Y // 2, TLOC, 2, Z, SC), np.float64)
        for y in range(0, Y, 2):
            ms = []
            for ph in range(NPH):
                acc = np.zeros((128, FREE), np.float64)
                for aidx, (ay, az) in enumerate(OFF9):
                    slab = u[:, ph, y + ay: y + ay + 2, az:az + Z, :].reshape(128, -1)
                    acc += wstat[:, ph * 9 + aidx, :].T.astype(np.float64) @ slab.astype(np.float64)
                ms.append(acc.reshape(128, 2, Z, SC))
            m = np.stack(ms, axis=0)  # (6, 128, 2, Z, SC)
            res = np.einsum("rp,pnyzs->nryzs", AT, m)  # (128, 4, 2, Z, SC)
            out[:, y // 2] = res
        results.append({"out": out})
    return results



# revision 2
# speedup vs baseline: 1.0759x; 1.0759x over previous
"""4D circular cross-correlation (qcd_ml C_Convolution, k=3, nd=4) on 8 TRN2 cores.

Math: out[o, x,y,z,t, s,c] = b[o] + sum_{i, ax,ay,az,at} W[i,o,ax,ay,az,at]
                                   * U[i, x+ax-1, y+ay-1, z+az-1, t+at-1, s,c]
(all site indices circular). U complex64 (4,16,16,16,32,4,3), W complex64
(4,4,3,3,3,3), b complex64 (4,).

Device mapping (per core, T sharded 8-way):
  - contraction (matmul partition) dim = (reim_in 2, C_in 4, X 16) = 128
  - output (PSUM partition) dim       = (reim_out 2, C_out 4, X0 16) = 128
  - X offsets (ax) live inside the stationary 128x128 matrices, circularly
    banded in (x, x0); complex arithmetic is the 2x2 [[Wr, Wi], [-Wi, Wr]]
    block over the reim axes.
  - T offsets (at) removed by a host-side Winograd F(4,3) transform along t
    (6 t-phases per 4 local t outputs).
  - Y offsets (ay) removed by a host-side Winograd F(2,3) transform along y
    (4 y-phases per y-pair): per (tph, yph) only the 3 az offsets remain as
    PSUM-accumulated matmuls.  PE work: 4 duos x 6 tph x 4 yph x 3 az = 288
    matmuls of 384 cols (vs 432 without the y transform).
  - moving free dim = (pair-in-duo 2, z 16, spin*color 12) = 384.
  - z circular handled by host padding to 18; t halo from neighbor T-slab;
    y halo inside the host B_y^T window.
  - On-device combine: A_y^T (4 ops, fused FD 6*384 across t-phases) then
    A_t^T (10 ops, FD 2*384) per duo, all fp16 SBUF operands so the DVE runs
    in 2x packed mode; Act drains PSUM->SBUF (fp16) per t-phase.  The duo's
    combine hides under the next duo's ~11.7us of matmuls.
  - Last duo computes t-phase 0 LAST (A_t column 0 = [1,0,0,0] -> only the
    t=0 output row depends on it), so nearly all combine work and 3/4 of the
    final store complete before the kernel's final matmul.
"""

import os
import sys
import itertools
import numpy as np

for _p in ("/opt/trn_rl_repo",):
    if _p not in sys.path and os.path.isdir(_p):
        sys.path.insert(0, _p)

C_IN, C_OUT = 4, 4
X = Y = Z = 16
T = 32
SC = 12  # spin*color
NCORES = 8
TLOC = T // NCORES          # 4 = one F(4,3) output tile
NPH = 6                     # Winograd F(4,3) t-phases
NYPH = 4                    # Winograd F(2,3) y-phases
NDUO = 4                    # 8 y-pairs in duos of 2
ZPAD = Z + 2                # 18
FREE = 2 * Z * SC           # 384: (pair-in-duo, z, sc)

# 16-bit data path: halves input DMA, halves LoadStationary (fp32r LS ~195ns
# would be the pipeline bottleneck; 16-bit LS ~97ns < 160ns matmul), and the
# fp16 SBUF combine runs the DVE at 2x packed mode. fp16 over bf16: same PE
# rate, 4x finer mantissa. Values are small (|U~|<~60, |wstat|<=4, combine
# intermediates <~40k < 65504) so fp16 range is safe.
CONV_DT = os.environ.get("CONV_DT", "fp16")

# Winograd F(4,3) along t, points [0,1,-1,2,-2,inf] (correlation form:
# out[r] = sum_k g[k] d[r+k], r=0..3, d = U[t0-1 .. t0+4]).
BT = np.array([
    [4, 0, -5, 0, 1, 0],
    [0, -4, -4, 1, 1, 0],
    [0, 4, -4, -1, 1, 0],
    [0, -2, -1, 2, 1, 0],
    [0, 2, -1, -2, 1, 0],
    [0, 4, 0, -5, 0, 1]], np.float64)
GT = np.array([
    [1 / 4, 0, 0],
    [-1 / 6, -1 / 6, -1 / 6],
    [-1 / 6, 1 / 6, -1 / 6],
    [1 / 24, 1 / 12, 1 / 6],
    [1 / 24, -1 / 12, 1 / 6],
    [0, 0, 1]], np.float64)
# A_t^T = [[1,1,1,1,1,0],
#          [0,1,-1,2,-2,0],
#          [0,1,1,4,4,0],
#          [0,1,-1,8,-8,1]]  -- applied on the device (DVE).

# Winograd F(2,3) along y, points [0,1,-1,inf] (correlation form:
# out[r] = sum_k g[k] d[r+k], r=0..1, d = U[y0-1 .. y0+2]).
BY = np.array([
    [1, 0, -1, 0],
    [0, 1, 1, 0],
    [0, -1, 1, 0],
    [0, 1, 0, -1]], np.float64)
GY = np.array([
    [1, 0, 0],
    [0.5, 0.5, 0.5],
    [0.5, -0.5, 0.5],
    [0, 0, 1]], np.float64)
# A_y^T = [[1,1,1,0],[0,1,-1,-1]] -- applied on the device (DVE).

# Per-duo t-phase compute order: tph 0 last, so the t-combine's only
# tph0-dependent output (t=0 row) is the single piece trailing the last
# matmul of the last duo.  mbuf slot s holds t-phase TORDER[s].
TORDER = (1, 2, 3, 4, 5, 0)
SLOT = {t: s for s, t in enumerate(TORDER)}


def _np_dt():
    if CONV_DT == "fp16":
        return np.dtype(np.float16)
    if CONV_DT == "bf16":
        import ml_dtypes
        return np.dtype(ml_dtypes.bfloat16)
    return np.dtype(np.float32)


def _prep_u_shards(U):
    """U complex (4,16,16,16,32,4,3) -> per-core arrays
    [128, NDUO, NPH, NYPH, 2, ZPAD, SC] of the (t,y)-Winograd field.

    Layout is (duo, tph)-chunk-major so the DMA stream matches compute
    order: one contiguous chunk per (duo, t-phase)."""
    dt = _np_dt()
    Ur = np.stack([U.real, U.imag], axis=0).astype(np.float64)  # (2,4,X,Y,Z,T,4,3)
    Ur = Ur.reshape(2, C_IN, X, Y, Z, T, SC)
    Upz = np.pad(Ur, ((0, 0), (0, 0), (0, 0), (0, 0), (1, 1), (0, 0), (0, 0)),
                 mode="wrap")  # z -> 18
    # y windows per pair: rows (2p-1 .. 2p+2) mod 16
    yidx = (2 * np.arange(Y // 2)[:, None] - 1 + np.arange(4)[None, :]) % Y
    shards = []
    for k in range(NCORES):
        t0 = k * TLOC
        tidx = np.arange(t0 - 1, t0 + 5) % T        # 6-point t window
        d = np.take(Upz, tidx, axis=5)              # (2,4,16,16,18,6,12)
        mt = np.einsum("pk,rixyzks->rixyzps", BT, d)  # tph: (2,4,16,16,18,6,12)
        dy = mt[:, :, :, yidx]                      # (2,4,16, pair8, j4, 18, 6, 12)
        m = np.einsum("qj,rixpjzts->rixptqzs", BY, dy)
        # (2,4,16, pair8, tph6, yph4, z18, s12) -> split pair into (duo, ind)
        m = m.reshape(2, C_IN, X, NDUO, 2, NPH, NYPH, ZPAD, SC)
        m = m.transpose(0, 1, 2, 3, 5, 6, 4, 7, 8)  # (...,duo,tph,yph,ind,z,s)
        m = m.reshape(128, NDUO, NPH, NYPH, 2, ZPAD, SC).astype(dt)
        shards.append(np.ascontiguousarray(m))
    return shards


def _prep_wstat(W):
    """W complex (4,4,3,3,3,3) -> [128, NPH, NYPH, 3, 128] stationary stack.

    Wg[tph,yph][i,o,ax,az] = sum_{at,ay} GT[tph,at] GY[yph,ay] W[i,o,ax,ay,az,at];
    band in (x,x0): x = (x0 + ax - 1) mod 16;
    ri block M = [[Wr, Wi], [-Wi, Wr]] (columns riO: out_r, out_i).
    """
    Wc = np.ascontiguousarray(W).astype(np.complex128)
    Wg = np.einsum("pt,qy,ioxyzt->pqioxz", GT.astype(np.complex128),
                   GY.astype(np.complex128), Wc)   # (6,4,4,4,3,3)
    stat = np.zeros((2, C_IN, X, NPH, NYPH, 3, 2, C_OUT, X), _np_dt())
    x0v = np.arange(X)
    for ph in range(NPH):
        for q in range(NYPH):
            for az in range(3):
                for ax in range(3):
                    wr = Wg[ph, q, :, :, ax, az].real.astype(np.float64)
                    wi = Wg[ph, q, :, :, ax, az].imag.astype(np.float64)
                    for x0 in range(X):
                        x = (x0 + ax - 1) % X
                        stat[0, :, x, ph, q, az, 0, :, x0] = wr
                        stat[1, :, x, ph, q, az, 0, :, x0] = -wi
                        stat[0, :, x, ph, q, az, 1, :, x0] = wi
                        stat[1, :, x, ph, q, az, 1, :, x0] = wr
    return np.ascontiguousarray(stat.reshape(128, NPH, NYPH, 3, 128))


def _assemble(results, b):
    """results[k]["out"]: [128, NDUO, TLOC, 2, 384] -> complex (4,16,16,16,32,4,3)."""
    out = np.empty((C_OUT, X, Y, Z, T, SC), np.complex64)
    for k in range(NCORES):
        r = np.asarray(results[k]["out"], np.float32).reshape(
            2, C_OUT, X, NDUO, TLOC, 2, 2, Z, SC)
        # axes: (ri, o, x, duo, t, yrow, ind, z, s); y = 4*duo + 2*ind + yrow
        r = r.transpose(0, 1, 2, 3, 6, 5, 7, 4, 8).reshape(
            2, C_OUT, X, Y, Z, TLOC, SC)
        out[:, :, :, :, k * TLOC:(k + 1) * TLOC, :] = r[0] + 1j * r[1]
    out += np.asarray(b, np.complex64).reshape(C_OUT, 1, 1, 1, 1, 1)
    return np.ascontiguousarray(out.reshape(C_OUT, X, Y, Z, T, 4, 3))


def _build_nc():
    import concourse.mybir as mybir
    from concourse import bacc, tile
    from contextlib import ExitStack

    f32 = mybir.dt.float32
    _dt16 = {"fp16": mybir.dt.float16, "bf16": mybir.dt.bfloat16}
    mm_dt = _dt16.get(CONV_DT, mybir.dt.float32r)
    out_dt = _dt16.get(CONV_DT, f32)
    cdt = out_dt  # combine dtype: 16-bit SBUF keeps the DVE in 2x mode
    AluOp = mybir.AluOpType

    nc = bacc.Bacc()
    w_dram = nc.declare_dram_parameter("wstat", [128, NPH, NYPH, 3, 128], mm_dt, isOutput=False)
    u_dram = nc.declare_dram_parameter("u", [128, NDUO, NPH, NYPH, 2, ZPAD, SC], mm_dt, isOutput=False)
    o_dram = nc.declare_dram_parameter("out", [128, NDUO, TLOC, 2, FREE], out_dt, isOutput=True)

    with tile.TileContext(nc) as tc, ExitStack() as ctx:
        ipool = ctx.enter_context(tc.tile_pool(name="inp", bufs=1))
        mpool = ctx.enter_context(tc.tile_pool(name="mb", bufs=2))
        ypool = ctx.enter_context(tc.tile_pool(name="yc", bufs=1))
        tpool = ctx.enter_context(tc.tile_pool(name="tc", bufs=1))
        opool = ctx.enter_context(tc.tile_pool(name="o", bufs=2))
        ppool = ctx.enter_context(tc.tile_pool(name="psum", bufs=2, space="PSUM"))

        wt = ipool.tile([128, NPH, NYPH, 3, 128], mm_dt, tag="w")
        ufull = ipool.tile([128, NDUO, NPH, NYPH, 2, ZPAD, SC], mm_dt, tag="u")
        # Consumption-ordered input streaming on a single SP queue (aggregate
        # DMA bandwidth is shared across queues; explicit order beats
        # arbitration).  First matmul gate: wt[1,0] + u[0,1,0] ~ 0.2 MB.
        nc.sync.dma_start(wt[:, 1, 0:1], w_dram[:, 1, 0:1])
        nc.sync.dma_start(ufull[:, 0, 1, 0:1], u_dram[:, 0, 1, 0:1])
        nc.sync.dma_start(wt[:, 1, 1:4], w_dram[:, 1, 1:4])
        nc.sync.dma_start(ufull[:, 0, 1, 1:4], u_dram[:, 0, 1, 1:4])
        for ph in TORDER[1:]:
            nc.sync.dma_start(wt[:, ph], w_dram[:, ph])
            nc.sync.dma_start(ufull[:, 0, ph], u_dram[:, 0, ph])
        for d in range(1, NDUO):
            for ph in TORDER:
                nc.sync.dma_start(ufull[:, d, ph], u_dram[:, d, ph])

        # PE warm-up: dummy matmuls on a zeroed scratch tile while the input
        # DMA streams; keeps the PE HAM at full clock when real work starts.
        warm = ipool.tile([128, FREE], mm_dt, tag="warm")
        nc.gpsimd.memset(warm[:], 0.0)
        wps = ppool.tile([128, NYPH, 512], f32, tag="pt")
        for _ in range(15):
            nc.tensor.matmul(wps[:, 0, 0:FREE], warm[:, 0:128], warm[:],
                             start=True, stop=True)

        def stt(out_ap, in0, scalar, in1, op1=AluOp.add):
            nc.vector.scalar_tensor_tensor(
                out_ap, in0=in0, scalar=scalar, in1=in1,
                op0=AluOp.mult, op1=op1)

        for d in range(NDUO):
            last = (d == NDUO - 1)
            mb = mpool.tile([128, NYPH, NPH, FREE], cdt, tag="mb")
            ot = opool.tile([128, TLOC, 2, FREE], out_dt, tag="ot")
            slots = TORDER[:5] if last else TORDER
            for s, ph in enumerate(slots):
                pt = ppool.tile([128, NYPH, 512], f32, tag="pt")
                for q in range(NYPH):
                    for az in range(3):
                        rhs = ufull[:, d, ph, q, :, az:az + Z, :]
                        nc.tensor.matmul(
                            pt[:, q, 0:FREE],
                            wt[:, ph, q, az, :],
                            rhs,
                            start=(az == 0),
                            stop=(az == 2),
                        )
                # PSUM->SBUF drain (fp16) on the Activation engine, off the
                # DVE critical path.
                nc.scalar.copy(mb[:, :, s, :], pt[:, :, 0:FREE])

            # A_y^T combine, fused across the drained t-phase slots:
            # row0 = m0+m1+m2, row1 = m1-m2-m3 (per y-pair).
            ns = 5 if last else 6
            s1 = ypool.tile([128, NPH, FREE], cdt, tag="s1")
            yb = ypool.tile([128, NPH, 2, FREE], cdt, tag="yb")
            dd = ypool.tile([128, NPH, FREE], cdt, tag="dd")
            nc.vector.tensor_add(s1[:, 0:ns], mb[:, 1, 0:ns], mb[:, 2, 0:ns])
            nc.vector.tensor_add(yb[:, 0:ns, 0, :], s1[:, 0:ns], mb[:, 0, 0:ns])
            nc.vector.tensor_sub(dd[:, 0:ns], mb[:, 1, 0:ns], mb[:, 2, 0:ns])
            nc.vector.tensor_sub(yb[:, 0:ns, 1, :], dd[:, 0:ns], mb[:, 3, 0:ns])

            # A_t^T combine; m_t lives at yb slot SLOT[t]. FD = 2*384.
            m1, m2, m3, m4, m5 = (yb[:, SLOT[t]] for t in (1, 2, 3, 4, 5))
            b_ = tpool.tile([128, 2, FREE], cdt, tag="b")
            nc.vector.tensor_add(b_[:], m1, m2)
            a_ = tpool.tile([128, 2, FREE], cdt, tag="a")
            stt(a_[:], m1, 2.0, b_[:], AluOp.subtract)
            u_ = tpool.tile([128, 2, FREE], cdt, tag="u")
            nc.vector.tensor_add(u_[:], m3, m4)
            sw = tpool.tile([128, 2, FREE], cdt, tag="sw")
            stt(sw[:], m3, 2.0, u_[:], AluOp.subtract)
            bu = tpool.tile([128, 2, FREE], cdt, tag="bu")
            nc.vector.tensor_add(bu[:], b_[:], u_[:])
            stt(ot[:, 1], sw[:], 2.0, a_[:])
            stt(ot[:, 2], u_[:], 4.0, b_[:])
            t3a = tpool.tile([128, 2, FREE], cdt, tag="t3a")
            stt(t3a[:], sw[:], 8.0, a_[:])
            nc.vector.tensor_add(ot[:, 3], t3a[:], m5)
            if not last:
                m0 = yb[:, SLOT[0]]
                nc.vector.tensor_add(ot[:, 0], bu[:], m0)
                nc.sync.dma_start(o_dram[:, d, 0:2], ot[:, 0:2])
                nc.sync.dma_start(o_dram[:, d, 2:4], ot[:, 2:4])
            else:
                # Rows 1..3 are tph0-free: store them, then run tph0's 12
                # matmuls and finish only the t=0 row behind them.
                nc.sync.dma_start(o_dram[:, d, 2:4], ot[:, 2:4])
                nc.sync.dma_start(o_dram[:, d, 1:2], ot[:, 1:2])
                pt = ppool.tile([128, NYPH, 512], f32, tag="pt")
                for q in range(NYPH):
                    for az in range(3):
                        rhs = ufull[:, d, 0, q, :, az:az + Z, :]
                        nc.tensor.matmul(
                            pt[:, q, 0:FREE],
                            wt[:, 0, q, az, :],
                            rhs,
                            start=(az == 0),
                            stop=(az == 2),
                        )
                nc.scalar.copy(mb[:, :, 5, :], pt[:, :, 0:FREE])
                nc.vector.tensor_add(s1[:, 5:6], mb[:, 1, 5:6], mb[:, 2, 5:6])
                nc.vector.tensor_add(yb[:, 5, 0, :], s1[:, 5], mb[:, 0, 5])
                nc.vector.tensor_sub(dd[:, 5:6], mb[:, 1, 5:6], mb[:, 2, 5:6])
                nc.vector.tensor_sub(yb[:, 5, 1, :], dd[:, 5], mb[:, 3, 5])
                nc.vector.tensor_add(ot[:, 0], bu[:], yb[:, 5])
                nc.sync.dma_start(o_dram[:, d, 0:1], ot[:, 0:1])

    # Bacc defers register allocation and sync-wait splitting to finalize();
    # run_bass_via_pjrt serializes the module as-is, so finalize here.
    nc.finalize()
    return nc


_NC_CACHE = None
LAST_RUN = None  # BassKernelResults of the most recent device run (for test.py)


def kernel(U, W, b):
    global _NC_CACHE, LAST_RUN
    shards = _prep_u_shards(np.asarray(U))
    wstat = _prep_wstat(np.asarray(W))

    if os.environ.get("CONV_EMULATE", "0") == "1":
        results = _emulate(shards, wstat)
    else:
        from concourse.bass_utils import run_bass_kernel_spmd
        if _NC_CACHE is None:
            _NC_CACHE = _build_nc()
        in_maps = [{"wstat": wstat, "u": u} for u in shards]
        trace = os.environ.get("CONV_TRACE", "0") == "1"
        LAST_RUN = run_bass_kernel_spmd(
            _NC_CACHE, in_maps, core_ids=list(range(NCORES)), trace=trace)
        results = LAST_RUN.results
    return _assemble(results, np.asarray(b))


def _emulate(shards, wstat):
    """Host-side emulation of the device program, mimicking the fp16
    rounding of the PSUM drain and each DVE combine op."""
    dt = _np_dt()

    def rnd(x):
        return x.astype(dt).astype(np.float64)

    results = []
    for u in shards:
        u = np.asarray(u, np.float64)
        w = np.asarray(wstat, np.float64)
        out = np.zeros((128, NDUO, TLOC, 2, FREE), np.float64)
        for d in range(NDUO):
            mbuf = np.zeros((128, NYPH, NPH, FREE), np.float64)
            for ph in range(NPH):
                for q in range(NYPH):
                    acc = np.zeros((128, FREE), np.float64)
                    for az in range(3):
                        slab = u[:, d, ph, q, :, az:az + Z, :].reshape(128, -1)
                        acc += w[:, ph, q, az, :].T @ slab
                    mbuf[:, q, SLOT[ph]] = rnd(acc)
            # A_y^T
            s1 = rnd(mbuf[:, 1] + mbuf[:, 2])
            yb0 = rnd(s1 + mbuf[:, 0])
            ddv = rnd(mbuf[:, 1] - mbuf[:, 2])
            yb1 = rnd(ddv - mbuf[:, 3])
            yb = np.stack([yb0, yb1], axis=2)  # (128, slot6, yrow2, FREE)
            m = {t: yb[:, SLOT[t]] for t in range(NPH)}
            b_ = rnd(m[1] + m[2])
            a_ = rnd(2.0 * m[1] - b_)
            u_ = rnd(m[3] + m[4])
            sw = rnd(2.0 * m[3] - u_)
            bu = rnd(b_ + u_)
            out[:, d, 0] = rnd(bu + m[0])
            out[:, d, 1] = rnd(2.0 * sw + a_)
            out[:, d, 2] = rnd(4.0 * u_ + b_)
            t3a = rnd(8.0 * sw + a_)
            out[:, d, 3] = rnd(t3a + m[5])
        results.append({"out": rnd(out)})
    return results


# revision 6
# speedup vs baseline: 1.1712x; 1.0886x over previous
"""4D circular cross-correlation (qcd_ml C_Convolution, k=3, nd=4) on 8 TRN2 cores.

Math: out[o, x,y,z,t, s,c] = b[o] + sum_{i, ax,ay,az,at} W[i,o,ax,ay,az,at]
                                   * U[i, x+ax-1, y+ay-1, z+az-1, t+at-1, s,c]
(all site indices circular). U complex64 (4,16,16,16,32,4,3), W complex64
(4,4,3,3,3,3), b complex64 (4,).

Device mapping (per core, T sharded 8-way):
  - contraction (matmul partition) dim = (reim_in 2, C_in 4, X 16) = 128
  - output (PSUM partition) dim       = (reim_out 2, C_out 4, X0 16) = 128
  - X offsets (ax) live inside the stationary 128x128 matrices, circularly
    banded in (x, x0); complex arithmetic is the 2x2 [[Wr, Wi], [-Wi, Wr]]
    block over the reim axes.
  - T offsets (at) removed by a host-side Winograd F(4,3) transform along t
    (6 t-phases per 4 local t outputs).
  - Y offsets (ay) removed by a host-side Winograd F(2,3) transform along y
    (4 y-phases per y-pair): per (tph, yph) only the 3 az offsets remain as
    PSUM-accumulated matmuls.  PE work: 4 duos x 6 tph x 4 yph x 3 az = 288
    matmuls of 384 cols (vs 432 without the y transform).
  - moving free dim = (pair-in-duo 2, z 16, spin*color 12) = 384.
  - z circular handled by host padding to 18; t halo from neighbor T-slab;
    y halo inside the host B_y^T window.
  - On-device combine: A_y^T (4 ops, fused FD 6*384 across t-phases) then
    A_t^T (10 ops, FD 2*384) per duo, all fp16 SBUF operands so the DVE runs
    in 2x packed mode; Act drains PSUM->SBUF (fp16) per t-phase.  The duo's
    combine hides under the next duo's ~11.7us of matmuls.
  - Last duo computes t-phase 0 LAST (A_t column 0 = [1,0,0,0] -> only the
    t=0 output row depends on it), so nearly all combine work and 3/4 of the
    final store complete before the kernel's final matmul.
"""

import os
import sys
import itertools
import numpy as np

for _p in ("/opt/trn_rl_repo",):
    if _p not in sys.path and os.path.isdir(_p):
        sys.path.insert(0, _p)

C_IN, C_OUT = 4, 4
X = Y = Z = 16
T = 32
SC = 12  # spin*color
NCORES = 8
TLOC = T // NCORES          # 4 = one F(4,3) output tile
NPH = 6                     # Winograd F(4,3) t-phases
NYPH = 4                    # Winograd F(2,3) y-phases
NDUO = 4                    # 8 y-pairs in duos of 2
ZPAD = Z + 2                # 18
FREE = 2 * Z * SC           # 384: (pair-in-duo, z, sc)

# 16-bit data path: halves input DMA, halves LoadStationary (fp32r LS ~195ns
# would be the pipeline bottleneck; 16-bit LS ~97ns < 160ns matmul), and the
# fp16 SBUF combine runs the DVE at 2x packed mode. fp16 over bf16: same PE
# rate, 4x finer mantissa. Values are small (|U~|<~60, |wstat|<=4, combine
# intermediates <~40k < 65504) so fp16 range is safe.
CONV_DT = os.environ.get("CONV_DT", "fp16")

# Winograd F(4,3) along t, points [0,1,-1,2,-2,inf] (correlation form:
# out[r] = sum_k g[k] d[r+k], r=0..3, d = U[t0-1 .. t0+4]).
BT = np.array([
    [4, 0, -5, 0, 1, 0],
    [0, -4, -4, 1, 1, 0],
    [0, 4, -4, -1, 1, 0],
    [0, -2, -1, 2, 1, 0],
    [0, 2, -1, -2, 1, 0],
    [0, 4, 0, -5, 0, 1]], np.float64)
GT = np.array([
    [1 / 4, 0, 0],
    [-1 / 6, -1 / 6, -1 / 6],
    [-1 / 6, 1 / 6, -1 / 6],
    [1 / 24, 1 / 12, 1 / 6],
    [1 / 24, -1 / 12, 1 / 6],
    [0, 0, 1]], np.float64)
# A_t^T = [[1,1,1,1,1,0],
#          [0,1,-1,2,-2,0],
#          [0,1,1,4,4,0],
#          [0,1,-1,8,-8,1]]  -- applied on the device (DVE).

# Winograd F(2,3) along y, points [0,1,-1,inf] (correlation form:
# out[r] = sum_k g[k] d[r+k], r=0..1, d = U[y0-1 .. y0+2]).
BY = np.array([
    [1, 0, -1, 0],
    [0, 1, 1, 0],
    [0, -1, 1, 0],
    [0, 1, 0, -1]], np.float64)
GY = np.array([
    [1, 0, 0],
    [0.5, 0.5, 0.5],
    [0.5, -0.5, 0.5],
    [0, 0, 1]], np.float64)
# A_y^T = [[1,1,1,0],[0,1,-1,-1]] -- applied on the device (DVE).

# Per-duo t-phase compute order: tph 0 last, so the t-combine's only
# tph0-dependent output (t=0 row) is the single piece trailing the last
# matmul of the last duo.  mbuf slot s holds t-phase TORDER[s].
TORDER = (1, 2, 3, 4, 5, 0)
SLOT = {t: s for s, t in enumerate(TORDER)}


def _np_dt():
    if CONV_DT == "fp16":
        return np.dtype(np.float16)
    if CONV_DT == "bf16":
        import ml_dtypes
        return np.dtype(ml_dtypes.bfloat16)
    return np.dtype(np.float32)


def _prep_u_shards(U):
    """U complex (4,16,16,16,32,4,3) -> per-core arrays
    [128, NDUO, NPH, NYPH, 2, ZPAD, SC] of the (t,y)-Winograd field.

    Layout is (duo, tph)-chunk-major so the DMA stream matches compute
    order: one contiguous chunk per (duo, t-phase)."""
    dt = _np_dt()
    Ur = np.stack([U.real, U.imag], axis=0).astype(np.float64)  # (2,4,X,Y,Z,T,4,3)
    Ur = Ur.reshape(2, C_IN, X, Y, Z, T, SC)
    Upz = np.pad(Ur, ((0, 0), (0, 0), (0, 0), (0, 0), (1, 1), (0, 0), (0, 0)),
                 mode="wrap")  # z -> 18
    # y windows per pair: rows (2p-1 .. 2p+2) mod 16
    yidx = (2 * np.arange(Y // 2)[:, None] - 1 + np.arange(4)[None, :]) % Y
    shards = []
    for k in range(NCORES):
        t0 = k * TLOC
        tidx = np.arange(t0 - 1, t0 + 5) % T        # 6-point t window
        d = np.take(Upz, tidx, axis=5)              # (2,4,16,16,18,6,12)
        mt = np.einsum("pk,rixyzks->rixyzps", BT, d)  # tph: (2,4,16,16,18,6,12)
        dy = mt[:, :, :, yidx]                      # (2,4,16, pair8, j4, 18, 6, 12)
        m = np.einsum("qj,rixpjzts->rixptqzs", BY, dy)
        # (2,4,16, pair8, tph6, yph4, z18, s12) -> split pair into (duo, ind)
        m = m.reshape(2, C_IN, X, NDUO, 2, NPH, NYPH, ZPAD, SC)
        m = m.transpose(0, 1, 2, 3, 5, 6, 4, 7, 8)  # (...,duo,tph,yph,ind,z,s)
        m = m.reshape(128, NDUO, NPH, NYPH, 2, ZPAD, SC).astype(dt)
        shards.append(np.ascontiguousarray(m))
    return shards


def _prep_wstat(W):
    """W complex (4,4,3,3,3,3) -> [128, NPH, NYPH, 3, 128] stationary stack.

    Wg[tph,yph][i,o,ax,az] = sum_{at,ay} GT[tph,at] GY[yph,ay] W[i,o,ax,ay,az,at];
    band in (x,x0): x = (x0 + ax - 1) mod 16;
    ri block M = [[Wr, Wi], [-Wi, Wr]] (columns riO: out_r, out_i).
    """
    Wc = np.ascontiguousarray(W).astype(np.complex128)
    Wg = np.einsum("pt,qy,ioxyzt->pqioxz", GT.astype(np.complex128),
                   GY.astype(np.complex128), Wc)   # (6,4,4,4,3,3)
    # Pre-scale t-phases 3,4 by 2: with m3'=2m3, m4'=2m4 the A_t combine
    # becomes t0=m0+p+r/2, t1=q+w, t2=p+2r, t3=q+4w+m5 (p,q,r,w = sums/
    # diffs of m1,m2 and m3',m4') — one more +-1 add, two fewer 1x-rate
    # scalar_tensor_tensor ops on the DVE.
    Wg[3] *= 2.0
    Wg[4] *= 2.0
    stat = np.zeros((2, C_IN, X, NPH, NYPH, 3, 2, C_OUT, X), _np_dt())
    x0v = np.arange(X)
    for ph in range(NPH):
        for q in range(NYPH):
            for az in range(3):
                for ax in range(3):
                    wr = Wg[ph, q, :, :, ax, az].real.astype(np.float64)
                    wi = Wg[ph, q, :, :, ax, az].imag.astype(np.float64)
                    for x0 in range(X):
                        x = (x0 + ax - 1) % X
                        stat[0, :, x, ph, q, az, 0, :, x0] = wr
                        stat[1, :, x, ph, q, az, 0, :, x0] = -wi
                        stat[0, :, x, ph, q, az, 1, :, x0] = wi
                        stat[1, :, x, ph, q, az, 1, :, x0] = wr
    return np.ascontiguousarray(stat.reshape(128, NPH, NYPH, 3, 128))


def _assemble(results, b):
    """results[k]["out"]: [128, NDUO, TLOC, 2, 384] -> complex (4,16,16,16,32,4,3)."""
    out = np.empty((C_OUT, X, Y, Z, T, SC), np.complex64)
    for k in range(NCORES):
        r = np.asarray(results[k]["out"], np.float32).reshape(
            2, C_OUT, X, NDUO, TLOC, 2, 2, Z, SC)
        # axes: (ri, o, x, duo, t, yrow, ind, z, s); y = 4*duo + 2*ind + yrow
        r = r.transpose(0, 1, 2, 3, 6, 5, 7, 4, 8).reshape(
            2, C_OUT, X, Y, Z, TLOC, SC)
        out[:, :, :, :, k * TLOC:(k + 1) * TLOC, :] = r[0] + 1j * r[1]
    out += np.asarray(b, np.complex64).reshape(C_OUT, 1, 1, 1, 1, 1)
    return np.ascontiguousarray(out.reshape(C_OUT, X, Y, Z, T, 4, 3))


def _build_nc():
    import concourse.mybir as mybir
    from concourse import bacc, tile
    from contextlib import ExitStack

    f32 = mybir.dt.float32
    _dt16 = {"fp16": mybir.dt.float16, "bf16": mybir.dt.bfloat16}
    mm_dt = _dt16.get(CONV_DT, mybir.dt.float32r)
    out_dt = _dt16.get(CONV_DT, f32)
    cdt = out_dt  # combine dtype: 16-bit SBUF keeps the DVE in 2x mode
    AluOp = mybir.AluOpType

    nc = bacc.Bacc()
    w_dram = nc.declare_dram_parameter("wstat", [128, NPH, NYPH, 3, 128], mm_dt, isOutput=False)
    u_dram = nc.declare_dram_parameter("u", [128, NDUO, NPH, NYPH, 2, ZPAD, SC], mm_dt, isOutput=False)
    o_dram = nc.declare_dram_parameter("out", [128, NDUO, TLOC, 2, FREE], out_dt, isOutput=True)

    with tile.TileContext(nc) as tc, ExitStack() as ctx:
        ipool = ctx.enter_context(tc.tile_pool(name="inp", bufs=1))
        mpool = ctx.enter_context(tc.tile_pool(name="mb", bufs=2))
        ypool = ctx.enter_context(tc.tile_pool(name="yc", bufs=1))
        tpool = ctx.enter_context(tc.tile_pool(name="tc", bufs=1))
        opool = ctx.enter_context(tc.tile_pool(name="o", bufs=2))
        ppool = ctx.enter_context(tc.tile_pool(name="psum", bufs=2, space="PSUM"))

        wt = ipool.tile([128, NPH, NYPH, 3, 128], mm_dt, tag="w")
        ufull = ipool.tile([128, NDUO, NPH, NYPH, 2, ZPAD, SC], mm_dt, tag="u")
        # Consumption-ordered input streaming on a single SP queue (aggregate
        # DMA bandwidth is shared across queues; explicit order beats
        # arbitration).  First matmul gate: wt[1,0] + u[0,1,0] ~ 0.2 MB.
        nc.sync.dma_start(wt[:, 1, 0:1], w_dram[:, 1, 0:1])
        nc.sync.dma_start(ufull[:, 0, 1, 0:1], u_dram[:, 0, 1, 0:1])
        nc.sync.dma_start(wt[:, 1, 1:4], w_dram[:, 1, 1:4])
        nc.sync.dma_start(ufull[:, 0, 1, 1:4], u_dram[:, 0, 1, 1:4])
        for ph in TORDER[1:]:
            nc.sync.dma_start(wt[:, ph], w_dram[:, ph])
            nc.sync.dma_start(ufull[:, 0, ph], u_dram[:, 0, ph])
        for d in range(1, NDUO):
            for ph in TORDER:
                nc.sync.dma_start(ufull[:, d, ph], u_dram[:, d, ph])

        # PE warm-up: dummy matmuls on a zeroed scratch tile while the input
        # DMA streams; keeps the PE HAM at full clock when real work starts.
        warm = ipool.tile([128, FREE], mm_dt, tag="warm")
        nc.gpsimd.memset(warm[:], 0.0)
        wps = ppool.tile([128, NYPH, 512], f32, tag="pt")
        for _ in range(15):
            nc.tensor.matmul(wps[:, 0, 0:FREE], warm[:, 0:128], warm[:],
                             start=True, stop=True)

        def stt(out_ap, in0, scalar, in1, op1=AluOp.add):
            nc.vector.scalar_tensor_tensor(
                out_ap, in0=in0, scalar=scalar, in1=in1,
                op0=AluOp.mult, op1=op1)

        # Optional offload of the four leading A_t adds to the (otherwise
        # idle) GpSimd engine; shares an SBUF port with the DVE, so A/B it.
        # (measured: gpsimd tensor_tensor on fp16 returns garbage and runs
        # 1.6-2.5us/op — keep everything on the DVE)
        gp = os.environ.get("CONV_GPOFF", "0") == "1"
        eng_pq = nc.gpsimd if gp else nc.vector

        def ycomb(mb, s1, yb, dd, lo, hi):
            # A_y^T over drained slots [lo:hi): row0 = m0+m1+m2,
            # row1 = m1-m2-m3 (per y-pair).
            nc.vector.tensor_add(s1[:, lo:hi], mb[:, 1, lo:hi], mb[:, 2, lo:hi])
            nc.vector.tensor_add(yb[:, lo:hi, 0, :], s1[:, lo:hi], mb[:, 0, lo:hi])
            nc.vector.tensor_sub(dd[:, lo:hi], mb[:, 1, lo:hi], mb[:, 2, lo:hi])
            nc.vector.tensor_sub(yb[:, lo:hi, 1, :], dd[:, lo:hi], mb[:, 3, lo:hi])

        for d in range(NDUO):
            last = (d == NDUO - 1)
            mb = mpool.tile([128, NYPH, NPH, FREE], cdt, tag="mb")
            ot = opool.tile([128, TLOC, 2, FREE], out_dt, tag="ot")
            s1 = ypool.tile([128, NPH, FREE], cdt, tag="s1")
            yb = ypool.tile([128, NPH, 2, FREE], cdt, tag="yb")
            dd = ypool.tile([128, NPH, FREE], cdt, tag="dd")
            slots = TORDER[:5] if last else TORDER
            for s, ph in enumerate(slots):
                pt = ppool.tile([128, NYPH, 512], f32, tag="pt")
                for q in range(NYPH):
                    for az in range(3):
                        rhs = ufull[:, d, ph, q, :, az:az + Z, :]
                        nc.tensor.matmul(
                            pt[:, q, 0:FREE],
                            wt[:, ph, q, az, :],
                            rhs,
                            start=(az == 0),
                            stop=(az == 2),
                        )
                # PSUM->SBUF drain (fp16) on the Activation engine, off the
                # DVE critical path.
                nc.scalar.copy(mb[:, :, s, :], pt[:, :, 0:FREE])
                if s == 2:
                    # Half-duo y-combine: lets the DVE start ~3 slots
                    # earlier instead of idling until the whole duo drains.
                    ycomb(mb, s1, yb, dd, 0, 3)
            ycomb(mb, s1, yb, dd, 3, 5 if last else 6)

            # A_t^T combine; m_t lives at yb slot SLOT[t]. FD = 2*384.
            # (m3,m4 arrive pre-scaled by 2 via the host weight scale.)
            m1, m2, m3, m4, m5 = (yb[:, SLOT[t]] for t in (1, 2, 3, 4, 5))
            p_ = tpool.tile([128, 2, FREE], cdt, tag="p")
            eng_pq.tensor_add(p_[:], m1, m2)
            q_ = tpool.tile([128, 2, FREE], cdt, tag="q")
            eng_pq.tensor_sub(q_[:], m1, m2)
            r_ = tpool.tile([128, 2, FREE], cdt, tag="r")
            eng_pq.tensor_add(r_[:], m3, m4)
            w_ = tpool.tile([128, 2, FREE], cdt, tag="w")
            eng_pq.tensor_sub(w_[:], m3, m4)
            bu = tpool.tile([128, 2, FREE], cdt, tag="bu")
            stt(bu[:], r_[:], 0.5, p_[:])          # p + r/2
            nc.vector.tensor_add(ot[:, 1], q_[:], w_[:])
            if not last:
                m0 = yb[:, SLOT[0]]
                nc.vector.tensor_add(ot[:, 0], bu[:], m0)
                nc.sync.dma_start(o_dram[:, d, 0:2], ot[:, 0:2])
            stt(ot[:, 2], r_[:], 2.0, p_[:])
            t3a = tpool.tile([128, 2, FREE], cdt, tag="t3a")
            stt(t3a[:], w_[:], 4.0, q_[:])
            nc.vector.tensor_add(ot[:, 3], t3a[:], m5)
            if not last:
                nc.sync.dma_start(o_dram[:, d, 2:4], ot[:, 2:4])
            else:
                # Rows 1..3 are tph0-free: store them, then run tph0's 12
                # matmuls and finish only the t=0 row behind them.
                nc.sync.dma_start(o_dram[:, d, 2:4], ot[:, 2:4])
                nc.sync.dma_start(o_dram[:, d, 1:2], ot[:, 1:2])
                pt = ppool.tile([128, NYPH, 512], f32, tag="pt")
                for q in range(NYPH):
                    for az in range(3):
                        rhs = ufull[:, d, 0, q, :, az:az + Z, :]
                        nc.tensor.matmul(
                            pt[:, q, 0:FREE],
                            wt[:, 0, q, az, :],
                            rhs,
                            start=(az == 0),
                            stop=(az == 2),
                        )
                nc.scalar.copy(mb[:, :, 5, :], pt[:, :, 0:FREE])
                ycomb(mb, s1, yb, dd, 5, 6)
                nc.vector.tensor_add(ot[:, 0], bu[:], yb[:, 5])
                nc.sync.dma_start(o_dram[:, d, 0:1], ot[:, 0:1])

    # Bacc defers register allocation and sync-wait splitting to finalize();
    # run_bass_via_pjrt serializes the module as-is, so finalize here.
    nc.finalize()
    return nc


_NC_CACHE = None
LAST_RUN = None  # BassKernelResults of the most recent device run (for test.py)


def kernel(U, W, b):
    global _NC_CACHE, LAST_RUN
    shards = _prep_u_shards(np.asarray(U))
    wstat = _prep_wstat(np.asarray(W))

    if os.environ.get("CONV_EMULATE", "0") == "1":
        results = _emulate(shards, wstat)
    else:
        from concourse.bass_utils import run_bass_kernel_spmd
        if _NC_CACHE is None:
            _NC_CACHE = _build_nc()
        in_maps = [{"wstat": wstat, "u": u} for u in shards]
        trace = os.environ.get("CONV_TRACE", "0") == "1"
        LAST_RUN = run_bass_kernel_spmd(
            _NC_CACHE, in_maps, core_ids=list(range(NCORES)), trace=trace)
        results = LAST_RUN.results
    return _assemble(results, np.asarray(b))


def _emulate(shards, wstat):
    """Host-side emulation of the device program, mimicking the fp16
    rounding of the PSUM drain and each DVE combine op."""
    dt = _np_dt()

    def rnd(x):
        return x.astype(dt).astype(np.float64)

    results = []
    for u in shards:
        u = np.asarray(u, np.float64)
        w = np.asarray(wstat, np.float64)
        out = np.zeros((128, NDUO, TLOC, 2, FREE), np.float64)
        for d in range(NDUO):
            mbuf = np.zeros((128, NYPH, NPH, FREE), np.float64)
            for ph in range(NPH):
                for q in range(NYPH):
                    acc = np.zeros((128, FREE), np.float64)
                    for az in range(3):
                        slab = u[:, d, ph, q, :, az:az + Z, :].reshape(128, -1)
                        acc += w[:, ph, q, az, :].T @ slab
                    mbuf[:, q, SLOT[ph]] = rnd(acc)
            # A_y^T
            s1 = rnd(mbuf[:, 1] + mbuf[:, 2])
            yb0 = rnd(s1 + mbuf[:, 0])
            ddv = rnd(mbuf[:, 1] - mbuf[:, 2])
            yb1 = rnd(ddv - mbuf[:, 3])
            yb = np.stack([yb0, yb1], axis=2)  # (128, slot6, yrow2, FREE)
            m = {t: yb[:, SLOT[t]] for t in range(NPH)}
            # A_t^T with m3,m4 pre-scaled by 2 in the weights.
            p_ = rnd(m[1] + m[2])
            q_ = rnd(m[1] - m[2])
            r_ = rnd(m[3] + m[4])
            w_ = rnd(m[3] - m[4])
            bu = rnd(0.5 * r_ + p_)
            out[:, d, 0] = rnd(bu + m[0])
            out[:, d, 1] = rnd(q_ + w_)
            out[:, d, 2] = rnd(2.0 * r_ + p_)
            t3a = rnd(4.0 * w_ + q_)
            out[:, d, 3] = rnd(t3a + m[5])
        results.append({"out": rnd(out)})
    return results
